# revision 12
# baseline (speedup 1.0000x reference)
"""Locally-connected conv (per-location weights) + ReLU on 8 Trainium2 cores.

Problem: x (B=64, Cin=64, H=64, W=64), weights (H, W, Cout=64, Cin=64, 3, 3)
  out[r,a,i,j] = relu( sum_{b,c,d} weights[i,j,a,b,c,d] * xpad[r,b,i+c,j+d] )

Sharding: data-parallel over H - core cid owns output rows i in [8*cid, 8*cid+8).

Matmul scheme (same PE structure as the 186us baseline, which is optimal for
this shape): per (row pair t, input column v, tap c=g), side A = row 2t runs
a 64x64 stationary at PE quadrant (0,0) (x plane 2t+g, partitions 0-63) and
side B = row 2t+1 at (64,64) (plane 2t+1+g, partitions 64-127); the two
quadrants stream weight columns concurrently (disjoint PE rows AND columns;
sharing rows serializes the streams - measured 172us vs 96us tensor time).

v3 improvements over the baseline:
  * fp8 weight storage: weights live in HBM as fp8 e3m4 (x128 scale),
    halving the dominant DMA traffic (38.9 -> 19.5 MB/core).  The PE cannot
    stream e3m4 (NRT_EXEC_UNIT_UNRECOVERABLE, found empirically; only
    e4m3/e5m2 stream, and e4m3's 3-bit mantissa gives 2.8e-2 > 2e-2 gate
    error), so Vector/GpSimd/Scalar upcast each chunk to bf16 in SBUF
    (partition-split ~ engine throughput), folding in the 1/128 descale.
  * x deduplication: HBM carries each padded x plane once; the plane-pair
    partition stacking (lower half = plane p, upper half = plane p+1) that
    side B needs is built on-chip by SBUF->SBUF DMA.  Saves ~4.3 MB/core.
  * Per-plane x tiles in a 6-deep ring: matmuls start after ~2 plane loads
    instead of after the whole x block; weight chunks are v-quartered for
    finer DMA/compute pipelining.
  * Drains split: ScalarE drains pslo while DVE drains pshi.

PSUM: row 2t accumulates [r, (j, a)] on partitions 0-63 over all 8 banks
(pslo = j<32, pshi = j>=32), row 2t+1 on partitions 64-127.  Each bank is
zero-initialized once per row pair by a full-bank matmul with zero weights
(start=True), so every real matmul just accumulates (start=False) and may
span any column window within a bank.  skip_group_check: the sim's global
group tracker mis-addresses base-partition-64 PSUM APs (its per-tensor
pending-zero model, which is what models HW, is correct).
"""

import ml_dtypes
import numpy as np

import concourse.bass as bass
import concourse.mybir as mybir
import concourse.tile as tile
from concourse import bacc
from concourse.bass_utils import run_bass_kernel_spmd

B = 64          # batch (= stationary M / out partitions per side)
CIN = 64        # in channels
COUT = 64       # out channels
H = 64
W = 64
KS = 3
NCORES = 8
RPC = H // NCORES        # output rows per core = 8
NT = RPC // 2            # row pairs per core = 4
NV = W + 2               # padded columns = 66
NPL = RPC + 2            # x planes per core = 10
QB = (0, 17, 33, 50, 66)  # v-quarter boundaries
NQ = 4
CHQ = 17 * 3 * COUT      # max weight cols per quarter-chunk = 3264
FP32 = mybir.dt.float32

# fp8 e3m4 storage: 4-bit mantissa suits N(0, 1/64^2) weights (measured
# 1.33e-2 max rel err vs 2.73e-2 for e4m3 on the real inputs).  Host
# pre-scales by 2^7 to center the distribution in e3m4's range; the upcast
# descales.
W_FP8 = True
WDT = mybir.dt.float8e3 if W_FP8 else mybir.dt.bfloat16
NP_WDT = ml_dtypes.float8_e3m4 if W_FP8 else ml_dtypes.bfloat16
W_SCALE = 128.0 if W_FP8 else 1.0
XDT = mybir.dt.bfloat16
NP_XDT = ml_dtypes.bfloat16
ODT = mybir.dt.bfloat16

# Upcast engines (measured on [128, 3264] fp8e3->bf16 chunks): Vector CAST
# 1.85us (tensor_copy fast path; tensor_scalar is a 46us microcode trap),
# Scalar ACTIVATE-Copy 3.0us, GpSimd CAST 11us (unused).  Vector takes 2 of
# 3 chunks per group, Scalar 1 (it also drains).  The 1/W_SCALE descale is
# folded into x on the host (bf16 is scale-invariant), so upcasts and
# drains are pure dtype converts / ReLUs.

_PROGRAM = None
LAST_RESULTS = None


def _segments(v):
    """PSUM j-window for input column v, split at bank (8-j) boundaries."""
    jlo, jhi = max(0, v - 2), min(W - 1, v)
    if jlo // 8 == jhi // 8:
        return [(jlo, jhi)]
    mid = 8 * (jhi // 8)
    return [(jlo, mid - 1), (mid, jhi)]


def _build_program():
    nc = bacc.Bacc("TRN2", target_bir_lowering=False, debug=False,
                   num_devices=NCORES)
    # xt[b, p, v, r]: padded x plane u0+p for channel b (planes stored once).
    xt = nc.dram_tensor("xt", [64, NPL, NV, B], XDT, kind="ExternalInput")
    # wp[t, g, q, k, col]: k<64 = (row 2t, c=g) weights over b, k>=64 =
    # (row 2t+1, c=g); col = (vv*3 + m)*64 + a with j = v-2+m, d = 2-m
    # (invalid j -> zero columns, never streamed).
    wp = nc.dram_tensor("wp", [NT, KS, NQ, 128, CHQ], WDT,
                        kind="ExternalInput")
    # ot[t, jhalf, (parity, r), (j%32, a)]
    ot = nc.dram_tensor("ot", [NT, 2, 128, 2048], ODT, kind="ExternalOutput")

    with tile.TileContext(nc) as tc:
        with (
            tc.tile_pool(name="xpool", bufs=6) as xpool,
            tc.tile_pool(name="wfpool", bufs=12) as wfpool,
            tc.tile_pool(name="wpool", bufs=12) as wpool,
            tc.tile_pool(name="opool", bufs=4) as opool,
            tc.tile_pool(name="zpool", bufs=1) as zpool,
            tc.tile_pool(name="pspool", bufs=1,
                         space=bass.MemorySpace.PSUM) as pspool,
        ):
            zt = zpool.tile([64, 512], XDT, tag="zt")
            nc.vector.memset(zt[:], 0.0)

            # xtiles[p][0:64] = plane u0+p, xtiles[p][64:128] = plane u0+p+1
            # (upper filled when plane p+1 loads; plane 9 goes straight from
            # HBM into xtiles[8]'s upper half - no tile of its own).
            xtiles = {}

            # x planes ride GpSimd's DMA queue so they never sit in front
            # of weight chunks in the Sync queue (DGE queues are FIFO; a
            # blocked head descriptor stalls everything behind it).
            def load_plane(p):
                if p == NPL - 1:
                    nc.gpsimd.dma_start(xtiles[p - 1][64:128], xt[:, p])
                    return
                xq = xpool.tile([128, NV, B], XDT, tag="xp")
                nc.gpsimd.dma_start(xq[0:64], xt[:, p])
                if p > 0:
                    nc.gpsimd.dma_start(xtiles[p - 1][64:128], xq[0:64])
                xtiles[p] = xq

            for p in range(4):
                load_plane(p)

            def fetch_chunk(t, g, q, ncols):
                """DMA one weight chunk; upcast to bf16 if stored fp8."""
                if not W_FP8:
                    wt = wpool.tile([128, CHQ], WDT, tag="w")
                    nc.sync.dma_start(wt[:, :ncols], wp[t, g, q, :, :ncols])
                    return wt
                wf = wfpool.tile([128, CHQ], WDT, tag="wf")
                nc.sync.dma_start(wf[:, :ncols], wp[t, g, q, :, :ncols])
                wt = wpool.tile([128, CHQ], XDT, tag="w")
                # g=0 arrives first: give it ScalarE (slower, overlaps the
                # later chunks' DMAs); Vector CAST handles g=1,2.
                if g == 0:
                    nc.scalar.activation(wt[:, :ncols], wf[:, :ncols],
                                         mybir.ActivationFunctionType.Copy)
                else:
                    nc.vector.tensor_copy(wt[:, :ncols], wf[:, :ncols])
                return wt

            for t in range(NT):
                pslo = pspool.tile([128, 2048], FP32, tag="pslo")
                pshi = pspool.tile([128, 2048], FP32, tag="pshi")
                for k in range(4):
                    nc.tensor.matmul(pslo[:, 512 * k:512 * (k + 1)],
                                     zt[:, 0:128], zt[:, 0:512],
                                     start=True, stop=False,
                                     skip_group_check=True)
                    nc.tensor.matmul(pshi[:, 512 * k:512 * (k + 1)],
                                     zt[:, 0:128], zt[:, 0:512],
                                     start=True, stop=False,
                                     skip_group_check=True)
                for q in range(NQ):
                    ncols = (QB[q + 1] - QB[q]) * 3 * COUT
                    wg = [fetch_chunk(t, g, q, ncols) for g in range(KS)]
                    if t == 0 and q == 0:
                        load_plane(4)
                        load_plane(5)
                    # g-major: the PE's in-order stream consumes chunk g
                    # fully before touching chunk g+1, so one late upcast
                    # never head-of-line-blocks the other chunks' matmuls.
                    for g in range(KS):
                        for vv in range(QB[q + 1] - QB[q]):
                            v = QB[q] + vv
                            segs = _segments(v)
                            for side in range(2):
                                pb = 64 * side
                                lhsT = xtiles[2 * t + g][pb:pb + 64, v, :]
                                for (j0, j1) in segs:
                                    m0 = j0 - v + 2
                                    n = (j1 - j0 + 1) * 64
                                    tgt = pslo if j0 < 32 else pshi
                                    c0 = (j0 % 32) * 64
                                    w0 = (vv * 3 + m0) * 64
                                    nc.tensor.matmul(
                                        tgt[pb:pb + 64, c0:c0 + n],
                                        lhsT, wg[g][pb:pb + 64, w0:w0 + n],
                                        start=False, stop=False,
                                        skip_group_check=True)
                for p in (2 * t + 6, 2 * t + 7):
                    if p < NPL:
                        load_plane(p)
                olo = opool.tile([128, 2048], ODT, tag="olo")
                ohi = opool.tile([128, 2048], ODT, tag="ohi")
                # Drains in halves: the next tile's bank-k zero matmul only
                # waits for the half-drain covering bank k, not the full 8KB.
                for h0 in (0, 1024):
                    nc.scalar.activation(olo[:, h0:h0 + 1024],
                                         pslo[:, h0:h0 + 1024],
                                         mybir.ActivationFunctionType.Relu)
                    nc.vector.tensor_relu(ohi[:, h0:h0 + 1024],
                                          pshi[:, h0:h0 + 1024])
                # Output DMAs ride the drain engines' own queues: pushed
                # right after the drain retires, so they never block weight
                # fetches in the Sync queue.
                nc.scalar.dma_start(ot[t, 0], olo[:])
                nc.scalar.dma_start(ot[t, 1], ohi[:])
    nc.compile()
    return nc


def _pack_weights(weights):
    """weights (i, j, a, b, c, d) fp32 -> F[i, c, b, v, m, a] with
    F[i,c,b,v,m,a] = W[i, v-2+m, a, b, c, 2-m] (zero at invalid j)."""
    T2 = weights.transpose(0, 4, 5, 2, 3, 1)[:, :, ::-1]  # [i, c, m, a, b, j]
    T2p = np.ascontiguousarray(np.pad(T2, [(0, 0)] * 5 + [(2, 2)]))
    s = T2p.strides
    E = np.lib.stride_tricks.as_strided(
        T2p, shape=(H, KS, NV, KS, COUT, CIN),
        strides=(s[0], s[1], s[5], s[2] + s[5], s[3], s[4]))
    # E[i, c, v, m, a, b] -> F[i, c, b, v, m, a]
    F = E.transpose(0, 1, 5, 2, 3, 4)
    if W_SCALE != 1.0:
        F = F * W_SCALE
    return F.astype(NP_WDT)


def _prep_x(x):
    xpad = np.pad(x, ((0, 0), (0, 0), (1, 1), (1, 1)))
    if W_SCALE != 1.0:
        xpad = xpad * (1.0 / W_SCALE)   # descale for the x128 weight scale
    return np.ascontiguousarray(xpad.transpose(1, 2, 3, 0)).astype(NP_XDT)


def _core_inputs(F, xf, cid):
    u0 = RPC * cid
    xt_core = np.ascontiguousarray(xf[:, u0:u0 + NPL])  # [64, 10, 66, 64]
    Fc = F[u0:u0 + RPC]                                 # [8, 3, 64, 66, 3, 64]
    wp_core = np.zeros((NT, KS, NQ, 128, CHQ), dtype=NP_WDT)
    for t in range(NT):
        for g in range(KS):
            wfull = np.concatenate([Fc[2 * t, g], Fc[2 * t + 1, g]],
                                   axis=0).reshape(128, -1)  # [128, 66*192]
            for q in range(NQ):
                c0, c1 = QB[q] * 192, QB[q + 1] * 192
                wp_core[t, g, q, :, :c1 - c0] = wfull[:, c0:c1]
    return {"xt": xt_core, "wp": wp_core}


def _unpack_out(o):
    o = np.asarray(o)                       # [4, 2, 128, 2048] bf16
    o = o.reshape(NT, 2, 2, B, 32, COUT)    # [t, jh, par, r, jr, a]
    o = o.transpose(3, 5, 0, 2, 1, 4)       # [r, a, t, par, jh, jr]
    return o.reshape(B, COUT, RPC, W)


def kernel(x, weights):
    global _PROGRAM, LAST_RESULTS
    x = np.ascontiguousarray(np.asarray(x, dtype=np.float32))
    weights = np.ascontiguousarray(np.asarray(weights, dtype=np.float32))
    assert x.shape == (B, CIN, H, W) and weights.shape == (H, W, COUT, CIN, KS, KS)

    F = _pack_weights(weights)      # [64, 3, 64, 66, 3, 64]
    xf = _prep_x(x)                 # [b, u, v, r] bf16, u in [0, 66)

    in_maps = [_core_inputs(F, xf, cid) for cid in range(NCORES)]

    if _PROGRAM is None:
        _PROGRAM = _build_program()
    res = run_bass_kernel_spmd(_PROGRAM, in_maps, list(range(NCORES)))
    LAST_RESULTS = res

    outs = [_unpack_out(res.results[c]["ot"]) for c in range(NCORES)]
    full = np.concatenate(outs, axis=2).astype(np.float32)
    return np.ascontiguousarray(full)


# revision 13
# speedup vs baseline: 1.1275x; 1.1275x over previous
"""Locally-connected conv (per-location weights) + ReLU on 8 Trainium2 cores.

Problem: x (B=64, Cin=64, H=64, W=64), weights (H, W, Cout=64, Cin=64, 3, 3)
  out[r,a,i,j] = relu( sum_{b,c,d} weights[i,j,a,b,c,d] * xpad[r,b,i+c,j+d] )

Sharding: data-parallel over H - core cid owns output rows i in [8*cid, 8*cid+8).

Matmul scheme (same PE structure as the 186us baseline, which is optimal for
this shape): per (row pair t, input column v, tap c=g), side A = row 2t runs
a 64x64 stationary at PE quadrant (0,0) (x plane 2t+g, partitions 0-63) and
side B = row 2t+1 at (64,64) (plane 2t+1+g, partitions 64-127); the two
quadrants stream weight columns concurrently (disjoint PE rows AND columns;
sharing rows serializes the streams - measured 172us vs 96us tensor time).

v3 improvements over the baseline:
  * fp8 weight storage: weights live in HBM as fp8 e3m4 (x128 scale),
    halving the dominant DMA traffic (38.9 -> 19.5 MB/core).  The PE cannot
    stream e3m4 (NRT_EXEC_UNIT_UNRECOVERABLE, found empirically; only
    e4m3/e5m2 stream, and e4m3's 3-bit mantissa gives 2.8e-2 > 2e-2 gate
    error), so Vector/GpSimd/Scalar upcast each chunk to bf16 in SBUF
    (partition-split ~ engine throughput), folding in the 1/128 descale.
  * x deduplication: HBM carries each padded x plane once; the plane-pair
    partition stacking (lower half = plane p, upper half = plane p+1) that
    side B needs is built on-chip by SBUF->SBUF DMA.  Saves ~4.3 MB/core.
  * Per-plane x tiles in a 6-deep ring: matmuls start after ~2 plane loads
    instead of after the whole x block; weight chunks are v-quartered for
    finer DMA/compute pipelining.
  * Drains split: ScalarE drains pslo while DVE drains pshi.

PSUM: row 2t accumulates [r, (j, a)] on partitions 0-63 over all 8 banks
(pslo = j<32, pshi = j>=32), row 2t+1 on partitions 64-127.  Each bank is
zero-initialized once per row pair by a full-bank matmul with zero weights
(start=True), so every real matmul just accumulates (start=False) and may
span any column window within a bank.  skip_group_check: the sim's global
group tracker mis-addresses base-partition-64 PSUM APs (its per-tensor
pending-zero model, which is what models HW, is correct).
"""

import ml_dtypes
import numpy as np

import concourse.bass as bass
import concourse.mybir as mybir
import concourse.tile as tile
from concourse import bacc
from concourse.bass_utils import run_bass_kernel_spmd

B = 64          # batch (= stationary M / out partitions per side)
CIN = 64        # in channels
COUT = 64       # out channels
H = 64
W = 64
KS = 3
NCORES = 8
RPC = H // NCORES        # output rows per core = 8
NT = RPC // 2            # row pairs per core = 4
NV = W + 2               # padded columns = 66
NPL = RPC + 2            # x planes per core = 10
QB = (0, 17, 33, 50, 66)  # v-quarter boundaries
NQ = 4
CHQ = 17 * 3 * COUT      # max weight cols per quarter-chunk = 3264
FP32 = mybir.dt.float32

# fp8 e3m4 storage: 4-bit mantissa suits N(0, 1/64^2) weights (measured
# 1.33e-2 max rel err vs 2.73e-2 for e4m3 on the real inputs).  Host
# pre-scales by 2^7 to center the distribution in e3m4's range; the upcast
# descales.
W_FP8 = True
WDT = mybir.dt.float8e3 if W_FP8 else mybir.dt.bfloat16
NP_WDT = ml_dtypes.float8_e3m4 if W_FP8 else ml_dtypes.bfloat16
W_SCALE = 128.0 if W_FP8 else 1.0
XDT = mybir.dt.bfloat16
NP_XDT = ml_dtypes.bfloat16
ODT = mybir.dt.bfloat16

# Upcast engines (measured on [128, 3264] fp8e3->bf16 chunks): Vector CAST
# 1.85us (tensor_copy fast path; tensor_scalar is a 46us microcode trap),
# Scalar ACTIVATE-Copy 3.0us, GpSimd CAST 11us (unused).  Vector takes 2 of
# 3 chunks per group, Scalar 1 (it also drains).  The 1/W_SCALE descale is
# folded into x on the host (bf16 is scale-invariant), so upcasts and
# drains are pure dtype converts / ReLUs.

_PROGRAM = None
LAST_RESULTS = None


def _segments(v):
    """PSUM j-window for input column v, split at bank (8-j) boundaries."""
    jlo, jhi = max(0, v - 2), min(W - 1, v)
    if jlo // 8 == jhi // 8:
        return [(jlo, jhi)]
    mid = 8 * (jhi // 8)
    return [(jlo, mid - 1), (mid, jhi)]


def _build_program():
    nc = bacc.Bacc("TRN2", target_bir_lowering=False, debug=False,
                   num_devices=NCORES)
    # xt[b, p, v, r]: padded x plane u0+p for channel b (planes stored once).
    xt = nc.dram_tensor("xt", [64, NPL, NV, B], XDT, kind="ExternalInput")
    # wp[t, g, q, k, col]: k<64 = (row 2t, c=g) weights over b, k>=64 =
    # (row 2t+1, c=g); col = (vv*3 + m)*64 + a with j = v-2+m, d = 2-m
    # (invalid j -> zero columns, never streamed).
    wp = nc.dram_tensor("wp", [NT, KS, NQ, 128, CHQ], WDT,
                        kind="ExternalInput")
    # ot[t, jhalf, (parity, r), (j%32, a)]
    ot = nc.dram_tensor("ot", [NT, 2, 128, 2048], ODT, kind="ExternalOutput")

    with tile.TileContext(nc) as tc:
        with (
            tc.tile_pool(name="xpool", bufs=6) as xpool,
            tc.tile_pool(name="wfpool", bufs=12) as wfpool,
            tc.tile_pool(name="wpool", bufs=12) as wpool,
            tc.tile_pool(name="opool", bufs=4) as opool,
            tc.tile_pool(name="zpool", bufs=1) as zpool,
            tc.tile_pool(name="pspool", bufs=1,
                         space=bass.MemorySpace.PSUM) as pspool,
        ):
            zt = zpool.tile([64, 512], XDT, tag="zt")
            nc.vector.memset(zt[:], 0.0)

            # xtiles[p][0:64] = plane u0+p, xtiles[p][64:128] = plane u0+p+1
            # (upper filled when plane p+1 loads; plane 9 goes straight from
            # HBM into xtiles[8]'s upper half - no tile of its own).
            xtiles = {}

            # x planes ride GpSimd's DMA queue so they never sit in front
            # of weight chunks in the Sync queue (DGE queues are FIFO; a
            # blocked head descriptor stalls everything behind it).
            def load_plane(p):
                if p == NPL - 1:
                    nc.gpsimd.dma_start(xtiles[p - 1][64:128], xt[:, p])
                    return
                xq = xpool.tile([128, NV, B], XDT, tag="xp")
                nc.gpsimd.dma_start(xq[0:64], xt[:, p])
                if p > 0:
                    nc.gpsimd.dma_start(xtiles[p - 1][64:128], xt[:, p])
                xtiles[p] = xq

            for p in range(4):
                load_plane(p)

            def fetch_chunk(t, g, q, ncols):
                """DMA one weight chunk; upcast to bf16 if stored fp8."""
                if not W_FP8:
                    wt = wpool.tile([128, CHQ], WDT, tag="w")
                    nc.sync.dma_start(wt[:, :ncols], wp[t, g, q, :, :ncols])
                    return wt
                wf = wfpool.tile([128, CHQ], WDT, tag="wf")
                nc.sync.dma_start(wf[:, :ncols], wp[t, g, q, :, :ncols])
                wt = wpool.tile([128, CHQ], XDT, tag="w")
                if g < 2:
                    nc.vector.tensor_copy(wt[:, :ncols], wf[:, :ncols])
                else:
                    nc.scalar.activation(wt[:, :ncols], wf[:, :ncols],
                                         mybir.ActivationFunctionType.Copy)
                return wt

            for t in range(NT):
                pslo = pspool.tile([128, 2048], FP32, tag="pslo")
                pshi = pspool.tile([128, 2048], FP32, tag="pshi")
                for k in range(4):
                    nc.tensor.matmul(pslo[:, 512 * k:512 * (k + 1)],
                                     zt[:, 0:128], zt[:, 0:512],
                                     start=True, stop=False,
                                     skip_group_check=True)
                    nc.tensor.matmul(pshi[:, 512 * k:512 * (k + 1)],
                                     zt[:, 0:128], zt[:, 0:512],
                                     start=True, stop=False,
                                     skip_group_check=True)
                for q in range(NQ):
                    ncols = (QB[q + 1] - QB[q]) * 3 * COUT
                    wg = [fetch_chunk(t, g, q, ncols) for g in range(KS)]
                    if t == 0 and q == 0:
                        load_plane(4)
                        load_plane(5)
                    for vv in range(QB[q + 1] - QB[q]):
                        v = QB[q] + vv
                        segs = _segments(v)
                        for g in range(KS):
                            for side in range(2):
                                pb = 64 * side
                                lhsT = xtiles[2 * t + g][pb:pb + 64, v, :]
                                for (j0, j1) in segs:
                                    m0 = j0 - v + 2
                                    n = (j1 - j0 + 1) * 64
                                    tgt = pslo if j0 < 32 else pshi
                                    c0 = (j0 % 32) * 64
                                    w0 = (vv * 3 + m0) * 64
                                    nc.tensor.matmul(
                                        tgt[pb:pb + 64, c0:c0 + n],
                                        lhsT, wg[g][pb:pb + 64, w0:w0 + n],
                                        start=False, stop=False,
                                        skip_group_check=True)
                for p in (2 * t + 6, 2 * t + 7):
                    if p < NPL:
                        load_plane(p)
                olo = opool.tile([128, 2048], ODT, tag="olo")
                ohi = opool.tile([128, 2048], ODT, tag="ohi")
                # Drains in halves: the next tile's bank-k zero matmul only
                # waits for the half-drain covering bank k, not the full 8KB.
                for h0 in (0, 1024):
                    nc.scalar.activation(olo[:, h0:h0 + 1024],
                                         pslo[:, h0:h0 + 1024],
                                         mybir.ActivationFunctionType.Relu)
                    nc.vector.tensor_relu(ohi[:, h0:h0 + 1024],
                                          pshi[:, h0:h0 + 1024])
                # Output DMAs ride the drain engines' own queues: pushed
                # right after the drain retires, so they never block weight
                # fetches in the Sync queue.
                nc.scalar.dma_start(ot[t, 0], olo[:])
                nc.scalar.dma_start(ot[t, 1], ohi[:])
    nc.compile()
    return nc


def _pack_weights(weights):
    """weights (i, j, a, b, c, d) fp32 -> F[i, c, b, v, m, a] with
    F[i,c,b,v,m,a] = W[i, v-2+m, a, b, c, 2-m] (zero at invalid j)."""
    T2 = weights.transpose(0, 4, 5, 2, 3, 1)[:, :, ::-1]  # [i, c, m, a, b, j]
    T2p = np.ascontiguousarray(np.pad(T2, [(0, 0)] * 5 + [(2, 2)]))
    s = T2p.strides
    E = np.lib.stride_tricks.as_strided(
        T2p, shape=(H, KS, NV, KS, COUT, CIN),
        strides=(s[0], s[1], s[5], s[2] + s[5], s[3], s[4]))
    # E[i, c, v, m, a, b] -> F[i, c, b, v, m, a]
    F = E.transpose(0, 1, 5, 2, 3, 4)
    if W_SCALE != 1.0:
        F = F * W_SCALE
    return F.astype(NP_WDT)


def _prep_x(x):
    xpad = np.pad(x, ((0, 0), (0, 0), (1, 1), (1, 1)))
    if W_SCALE != 1.0:
        xpad = xpad * (1.0 / W_SCALE)   # descale for the x128 weight scale
    return np.ascontiguousarray(xpad.transpose(1, 2, 3, 0)).astype(NP_XDT)


def _core_inputs(F, xf, cid):
    u0 = RPC * cid
    xt_core = np.ascontiguousarray(xf[:, u0:u0 + NPL])  # [64, 10, 66, 64]
    Fc = F[u0:u0 + RPC]                                 # [8, 3, 64, 66, 3, 64]
    wp_core = np.zeros((NT, KS, NQ, 128, CHQ), dtype=NP_WDT)
    for t in range(NT):
        for g in range(KS):
            wfull = np.concatenate([Fc[2 * t, g], Fc[2 * t + 1, g]],
                                   axis=0).reshape(128, -1)  # [128, 66*192]
            for q in range(NQ):
                c0, c1 = QB[q] * 192, QB[q + 1] * 192
                wp_core[t, g, q, :, :c1 - c0] = wfull[:, c0:c1]
    return {"xt": xt_core, "wp": wp_core}


def _unpack_out(o):
    o = np.asarray(o)                       # [4, 2, 128, 2048] bf16
    o = o.reshape(NT, 2, 2, B, 32, COUT)    # [t, jh, par, r, jr, a]
    o = o.transpose(3, 5, 0, 2, 1, 4)       # [r, a, t, par, jh, jr]
    return o.reshape(B, COUT, RPC, W)


def kernel(x, weights):
    global _PROGRAM, LAST_RESULTS
    x = np.ascontiguousarray(np.asarray(x, dtype=np.float32))
    weights = np.ascontiguousarray(np.asarray(weights, dtype=np.float32))
    assert x.shape == (B, CIN, H, W) and weights.shape == (H, W, COUT, CIN, KS, KS)

    F = _pack_weights(weights)      # [64, 3, 64, 66, 3, 64]
    xf = _prep_x(x)                 # [b, u, v, r] bf16, u in [0, 66)

    in_maps = [_core_inputs(F, xf, cid) for cid in range(NCORES)]

    if _PROGRAM is None:
        _PROGRAM = _build_program()
    res = run_bass_kernel_spmd(_PROGRAM, in_maps, list(range(NCORES)))
    LAST_RESULTS = res

    outs = [_unpack_out(res.results[c]["ot"]) for c in range(NCORES)]
    full = np.concatenate(outs, axis=2).astype(np.float32)
    return np.ascontiguousarray(full)


# revision 15
# speedup vs baseline: 1.1461x; 1.0165x over previous
"""Locally-connected conv (per-location weights) + ReLU on 8 Trainium2 cores.

Problem: x (B=64, Cin=64, H=64, W=64), weights (H, W, Cout=64, Cin=64, 3, 3)
  out[r,a,i,j] = relu( sum_{b,c,d} weights[i,j,a,b,c,d] * xpad[r,b,i+c,j+d] )

Sharding: data-parallel over H - core cid owns output rows i in [8*cid, 8*cid+8).

Matmul scheme (same PE structure as the 186us baseline, which is optimal for
this shape): per (row pair t, input column v, tap c=g), side A = row 2t runs
a 64x64 stationary at PE quadrant (0,0) (x plane 2t+g, partitions 0-63) and
side B = row 2t+1 at (64,64) (plane 2t+1+g, partitions 64-127); the two
quadrants stream weight columns concurrently (disjoint PE rows AND columns;
sharing rows serializes the streams - measured 172us vs 96us tensor time).

Improvements over the 186us baseline (measured 150us):
  * fp8 weight storage: weights live in HBM as fp8 e3m4 (x128 scale),
    halving the dominant DMA traffic (38.9 -> 19.5 MB/core).  The PE cannot
    stream e3m4 (NRT_EXEC_UNIT_UNRECOVERABLE, found empirically; only
    e4m3/e5m2 stream, and e4m3's 3-bit mantissa gives 2.8e-2 > 2e-2 gate
    error), so each chunk is upcast to bf16 in SBUF: Vector CAST
    (tensor_copy) for taps g=0,1 and ScalarE ACTIVATE-Copy for g=2.
    The x128 descale is folded into x on the host (bf16 is scale-invariant
    so x/128 is exact), keeping upcasts and drains pure converts/ReLUs.
  * x deduplication: HBM carries each padded x plane once; the plane-pair
    partition stacking (lower half = plane p, upper half = plane p+1) that
    side B needs is built on-chip by SBUF->SBUF DMA.  Saves ~4.3 MB/core.
  * Per-plane x tiles in a 6-deep ring: matmuls start after ~2 plane loads
    instead of after the whole x block; weight chunks are v-quartered for
    finer DMA/compute pipelining.
  * Drains split: ScalarE drains pslo while DVE drains pshi.

Measured traps baked into this design (do not "simplify" these away):
  * DVE tensor_scalar with fp8 input is a 46us-per-chunk microcode trap;
    tensor_copy (CAST, 1.85us) and tensor_tensor are the fast paths.
  * K=128 tap-pairing (stacking taps c=0,1 in the contraction) serializes
    the even/odd-row streams (PE tiles sharing rows share the row input
    port): tensor time 172us vs 96us for the diagonal 64x64 scheme.
  * Emitting matmuls g-major (chunk-major) instead of v-major slows the
    stream ~20% (575us vs 475us summed matmul time).
  * gpsimd dma accum (software DGE) does not dtype-convert; fp8 DMA-add
    into a bf16 tile produces garbage.

PSUM: row 2t accumulates [r, (j, a)] on partitions 0-63 over all 8 banks
(pslo = j<32, pshi = j>=32), row 2t+1 on partitions 64-127.  Each bank is
zero-initialized once per row pair by a full-bank matmul with zero weights
(start=True), so every real matmul just accumulates (start=False) and may
span any column window within a bank.  skip_group_check: the sim's global
group tracker mis-addresses base-partition-64 PSUM APs (its per-tensor
pending-zero model, which is what models HW, is correct).
"""

import ml_dtypes
import numpy as np

import concourse.bass as bass
import concourse.mybir as mybir
import concourse.tile as tile
from concourse import bacc
from concourse.bass_utils import run_bass_kernel_spmd

B = 64          # batch (= stationary M / out partitions per side)
CIN = 64        # in channels
COUT = 64       # out channels
H = 64
W = 64
KS = 3
NCORES = 8
RPC = H // NCORES        # output rows per core = 8
NT = RPC // 2            # row pairs per core = 4
NV = W + 2               # padded columns = 66
NPL = RPC + 2            # x planes per core = 10
QB = (0, 17, 33, 50, 66)  # v-quarter boundaries
NQ = 4
CHQ = 17 * 3 * COUT      # max weight cols per quarter-chunk = 3264
FP32 = mybir.dt.float32

# fp8 e3m4 storage: 4-bit mantissa suits N(0, 1/64^2) weights (measured
# 1.33e-2 max rel err vs 2.73e-2 for e4m3 on the real inputs).  Host
# pre-scales by 2^7 to center the distribution in e3m4's range; the upcast
# descales.
W_FP8 = True
WDT = mybir.dt.float8e3 if W_FP8 else mybir.dt.bfloat16
NP_WDT = ml_dtypes.float8_e3m4 if W_FP8 else ml_dtypes.bfloat16
W_SCALE = 128.0 if W_FP8 else 1.0
XDT = mybir.dt.bfloat16
NP_XDT = ml_dtypes.bfloat16
ODT = mybir.dt.bfloat16

# Upcast engines (measured on [128, 3264] fp8e3->bf16 chunks): Vector CAST
# 1.85us (tensor_copy fast path; tensor_scalar is a 46us microcode trap),
# Scalar ACTIVATE-Copy 3.0us, GpSimd CAST 11us (unused).  Vector takes 2 of
# 3 chunks per group, Scalar 1 (it also drains).  The 1/W_SCALE descale is
# folded into x on the host (bf16 is scale-invariant), so upcasts and
# drains are pure dtype converts / ReLUs.

_PROGRAM = None
LAST_RESULTS = None


def _segments(v):
    """PSUM j-window for input column v, split at bank (8-j) boundaries."""
    jlo, jhi = max(0, v - 2), min(W - 1, v)
    if jlo // 8 == jhi // 8:
        return [(jlo, jhi)]
    mid = 8 * (jhi // 8)
    return [(jlo, mid - 1), (mid, jhi)]


def _build_program():
    nc = bacc.Bacc("TRN2", target_bir_lowering=False, debug=False,
                   num_devices=NCORES)
    # xt[b, p, v, r]: padded x plane u0+p for channel b (planes stored once).
    xt = nc.dram_tensor("xt", [64, NPL, NV, B], XDT, kind="ExternalInput")
    # wp[t, g, q, k, col]: k<64 = (row 2t, c=g) weights over b, k>=64 =
    # (row 2t+1, c=g); col = (vv*3 + m)*64 + a with j = v-2+m, d = 2-m
    # (invalid j -> zero columns, never streamed).
    wp = nc.dram_tensor("wp", [NT, KS, NQ, 128, CHQ], WDT,
                        kind="ExternalInput")
    # ot[t, jhalf, (parity, r), (j%32, a)]
    ot = nc.dram_tensor("ot", [NT, 2, 128, 2048], ODT, kind="ExternalOutput")

    with tile.TileContext(nc) as tc:
        with (
            tc.tile_pool(name="xpool", bufs=6) as xpool,
            tc.tile_pool(name="wfpool", bufs=9) as wfpool,
            tc.tile_pool(name="wpool", bufs=9) as wpool,
            tc.tile_pool(name="opool", bufs=4) as opool,
            tc.tile_pool(name="zpool", bufs=1) as zpool,
            tc.tile_pool(name="pspool", bufs=1,
                         space=bass.MemorySpace.PSUM) as pspool,
        ):
            zt = zpool.tile([64, 512], XDT, tag="zt")
            nc.vector.memset(zt[:], 0.0)

            # xtiles[p][0:64] = plane u0+p, xtiles[p][64:128] = plane u0+p+1
            # (upper filled when plane p+1 loads; plane 9 goes straight from
            # HBM into xtiles[8]'s upper half - no tile of its own).
            xtiles = {}

            def load_plane(p):
                if p == NPL - 1:
                    nc.sync.dma_start(xtiles[p - 1][64:128], xt[:, p])
                    return
                xq = xpool.tile([128, NV, B], XDT, tag="xp")
                nc.sync.dma_start(xq[0:64], xt[:, p])
                if p > 0:
                    nc.sync.dma_start(xtiles[p - 1][64:128], xq[0:64])
                xtiles[p] = xq

            for p in range(6):
                load_plane(p)

            def fetch_chunk(t, g, q, ncols):
                """DMA one weight chunk; upcast to bf16 if stored fp8."""
                if not W_FP8:
                    wt = wpool.tile([128, CHQ], WDT, tag="w")
                    nc.sync.dma_start(wt[:, :ncols], wp[t, g, q, :, :ncols])
                    return wt
                wf = wfpool.tile([128, CHQ], WDT, tag="wf")
                nc.sync.dma_start(wf[:, :ncols], wp[t, g, q, :, :ncols])
                wt = wpool.tile([128, CHQ], XDT, tag="w")
                if g < 2:
                    nc.vector.tensor_copy(wt[:, :ncols], wf[:, :ncols])
                else:
                    nc.scalar.activation(wt[:, :ncols], wf[:, :ncols],
                                         mybir.ActivationFunctionType.Copy)
                return wt

            for t in range(NT):
                pslo = pspool.tile([128, 2048], FP32, tag="pslo")
                pshi = pspool.tile([128, 2048], FP32, tag="pshi")
                for k in range(4):
                    nc.tensor.matmul(pslo[:, 512 * k:512 * (k + 1)],
                                     zt[:, 0:128], zt[:, 0:512],
                                     start=True, stop=False,
                                     skip_group_check=True)
                    nc.tensor.matmul(pshi[:, 512 * k:512 * (k + 1)],
                                     zt[:, 0:128], zt[:, 0:512],
                                     start=True, stop=False,
                                     skip_group_check=True)
                for q in range(NQ):
                    ncols = (QB[q + 1] - QB[q]) * 3 * COUT
                    wg = [fetch_chunk(t, g, q, ncols) for g in range(KS)]
                    for vv in range(QB[q + 1] - QB[q]):
                        v = QB[q] + vv
                        segs = _segments(v)
                        for g in range(KS):
                            for side in range(2):
                                pb = 64 * side
                                lhsT = xtiles[2 * t + g][pb:pb + 64, v, :]
                                for (j0, j1) in segs:
                                    m0 = j0 - v + 2
                                    n = (j1 - j0 + 1) * 64
                                    tgt = pslo if j0 < 32 else pshi
                                    c0 = (j0 % 32) * 64
                                    w0 = (vv * 3 + m0) * 64
                                    nc.tensor.matmul(
                                        tgt[pb:pb + 64, c0:c0 + n],
                                        lhsT, wg[g][pb:pb + 64, w0:w0 + n],
                                        start=False, stop=False,
                                        skip_group_check=True)
                for p in (2 * t + 6, 2 * t + 7):
                    if p < NPL:
                        load_plane(p)
                olo = opool.tile([128, 2048], ODT, tag="olo")
                ohi = opool.tile([128, 2048], ODT, tag="ohi")
                nc.scalar.activation(olo[:], pslo[:],
                                     mybir.ActivationFunctionType.Relu)
                nc.vector.tensor_relu(ohi[:], pshi[:])
                nc.sync.dma_start(ot[t, 0], olo[:])
                nc.sync.dma_start(ot[t, 1], ohi[:])
    nc.compile()
    return nc


def _pack_weights(weights):
    """weights (i, j, a, b, c, d) fp32 -> F[i, c, b, v, m, a] with
    F[i,c,b,v,m,a] = W[i, v-2+m, a, b, c, 2-m] (zero at invalid j)."""
    T2 = weights.transpose(0, 4, 5, 2, 3, 1)[:, :, ::-1]  # [i, c, m, a, b, j]
    T2p = np.ascontiguousarray(np.pad(T2, [(0, 0)] * 5 + [(2, 2)]))
    s = T2p.strides
    E = np.lib.stride_tricks.as_strided(
        T2p, shape=(H, KS, NV, KS, COUT, CIN),
        strides=(s[0], s[1], s[5], s[2] + s[5], s[3], s[4]))
    # E[i, c, v, m, a, b] -> F[i, c, b, v, m, a]
    F = E.transpose(0, 1, 5, 2, 3, 4)
    if W_SCALE != 1.0:
        F = F * W_SCALE
    return F.astype(NP_WDT)


def _prep_x(x):
    xpad = np.pad(x, ((0, 0), (0, 0), (1, 1), (1, 1)))
    if W_SCALE != 1.0:
        xpad = xpad * (1.0 / W_SCALE)   # descale for the x128 weight scale
    return np.ascontiguousarray(xpad.transpose(1, 2, 3, 0)).astype(NP_XDT)


def _core_inputs(F, xf, cid):
    u0 = RPC * cid
    xt_core = np.ascontiguousarray(xf[:, u0:u0 + NPL])  # [64, 10, 66, 64]
    Fc = F[u0:u0 + RPC]                                 # [8, 3, 64, 66, 3, 64]
    wp_core = np.zeros((NT, KS, NQ, 128, CHQ), dtype=NP_WDT)
    for t in range(NT):
        for g in range(KS):
            wfull = np.concatenate([Fc[2 * t, g], Fc[2 * t + 1, g]],
                                   axis=0).reshape(128, -1)  # [128, 66*192]
            for q in range(NQ):
                c0, c1 = QB[q] * 192, QB[q + 1] * 192
                wp_core[t, g, q, :, :c1 - c0] = wfull[:, c0:c1]
    return {"xt": xt_core, "wp": wp_core}


def _unpack_out(o):
    o = np.asarray(o)                       # [4, 2, 128, 2048] bf16
    o = o.reshape(NT, 2, 2, B, 32, COUT)    # [t, jh, par, r, jr, a]
    o = o.transpose(3, 5, 0, 2, 1, 4)       # [r, a, t, par, jh, jr]
    return o.reshape(B, COUT, RPC, W)


def kernel(x, weights):
    global _PROGRAM, LAST_RESULTS
    x = np.ascontiguousarray(np.asarray(x, dtype=np.float32))
    weights = np.ascontiguousarray(np.asarray(weights, dtype=np.float32))
    assert x.shape == (B, CIN, H, W) and weights.shape == (H, W, COUT, CIN, KS, KS)

    F = _pack_weights(weights)      # [64, 3, 64, 66, 3, 64]
    xf = _prep_x(x)                 # [b, u, v, r] bf16, u in [0, 66)

    in_maps = [_core_inputs(F, xf, cid) for cid in range(NCORES)]

    if _PROGRAM is None:
        _PROGRAM = _build_program()
    res = run_bass_kernel_spmd(_PROGRAM, in_maps, list(range(NCORES)))
    LAST_RESULTS = res

    outs = [_unpack_out(res.results[c]["ot"]) for c in range(NCORES)]
    full = np.concatenate(outs, axis=2).astype(np.float32)
    return np.ascontiguousarray(full)
